# revision 9
# baseline (speedup 1.0000x reference)
"""LoRA linear layer (out = x @ (W + B@A).T + bias) on 8 trn2 NeuronCores.

Strategy: data-parallel over tokens (B*S = 8192 -> 1024 tokens/core).
Each core computes its token-shard against the full weight:
  - x shard is DMA'd in, transposed on the PE (128x128 tiles) into a
    resident SBUF xT [d_in, T] laid out as [128, KT, T].
  - U = (x @ A.T).T = [r, T] computed once with A.T as stationary operand.
  - For each 128-row block m of W: DMA the rows, PE-transpose into
    WT [128, KT, 128]; accumulate psum[o=128, t=512] over the 32 k-tiles
    (fp32r matmuls, weight loaded once per (m,k) and reused for both
    t-chunks), then one rank-16 matmul adds the LoRA term from B.T and U.
  - psum is evicted through the Scalar engine with the bias added
    (bias laid out per-partition), PE-transposed back to [t, o] tiles and
    DMA'd out contiguously.
"""

import sys
import types

sys.path.insert(0, "/opt/trn_rl_repo")

import numpy as np

import concourse.bass as bass  # noqa: F401
import concourse.bacc as bacc
import concourse.tile as tile
from concourse import mybir, bass_utils
from concourse.masks import make_identity
from contextlib import ExitStack

P = 128
N_CORES = 8

# Full problem shapes (hardcoded per contract).
B_FULL, S_FULL, D_IN, D_OUT, R = 4, 2048, 4096, 4096, 16
T_CORE = (B_FULL * S_FULL) // N_CORES  # 1024 tokens per core


def build_nc(T=T_CORE, DIN=D_IN, DOUT=D_OUT, r=R, tr_fpr=False, w_bf16=False):
    """Build the per-core bass program. All cores run the same program on
    different token shards."""
    FP = mybir.dt.float32
    FPR = mybir.dt.float32r
    KT = DIN // P
    MT = DOUT // P
    NCH = min(512, T)  # moving-operand chunk (>=256 keeps fp32r at full rate)
    NT = T // NCH
    TG = 4  # transposes grouped per PSUM bank before one batched eviction
    NWCH = 4  # W row-block DMA'd in this many chunks
    HKT = KT // NWCH

    nc = bacc.Bacc("TRN2", target_bir_lowering=False, debug=False)
    x_d = nc.dram_tensor("x", [T, DIN], FP, kind="ExternalInput").ap()
    w_d = nc.dram_tensor("w", [DOUT, DIN], FP, kind="ExternalInput").ap()
    br_d = nc.dram_tensor("bias_r", [P, MT], FP, kind="ExternalInput").ap()
    at_d = nc.dram_tensor("at", [DIN, r], FP, kind="ExternalInput").ap()
    bt_d = nc.dram_tensor("bt", [r, DOUT], FP, kind="ExternalInput").ap()
    out_d = nc.dram_tensor("out", [T, DOUT], FP, kind="ExternalOutput").ap()

    with tile.TileContext(nc) as tc, ExitStack() as ctx:
        const = ctx.enter_context(tc.tile_pool(name="const", bufs=1))
        ident = const.tile([P, P], FP)
        make_identity(nc, ident)
        bias_sb = const.tile([P, MT], FP)
        nc.sync.dma_start(bias_sb[:], br_d[:])
        at_sb = const.tile([P, KT, r], FPR)
        bt_sb = const.tile([r, DOUT], FPR)
        xt_all = const.tile([P, KT, T], FPR)  # resident x^T, 16 MB
        u_sb = const.tile([r, T], FPR)

        tp_psum = ctx.enter_context(tc.tile_pool(name="tpps", bufs=2, space="PSUM"))
        ot_psum = ctx.enter_context(tc.tile_pool(name="otps", bufs=2, space="PSUM"))

        # ---- stage 1: transpose x shard into xt_all (TG tiles per bank) ----
        with tc.tile_pool(name="xrawp", bufs=2) as xraw_pool:
            at_raw = xraw_pool.tile([P, KT, r], FP, tag="xraw")
            nc.sync.dma_start(at_raw[:], at_d.rearrange("(k p) r -> p k r", p=P))
            nc.vector.tensor_copy(at_sb[:], at_raw[:])
            bt_raw = xraw_pool.tile([r, DOUT], FP, tag="xraw")
            nc.sync.dma_start(bt_raw[:], bt_d[:])
            nc.vector.tensor_copy(bt_sb[:], bt_raw[:])
            TBN = NCH // P  # token-blocks per U chunk
            for tb in range(T // P):
                tsl = slice(tb * P, (tb + 1) * P)
                xraw = xraw_pool.tile([P, DIN], FP, tag="xraw")
                QD = DIN // 4
                for q in range(4):
                    nc.sync.dma_start(
                        xraw[:, q * QD : (q + 1) * QD],
                        x_d[tsl, q * QD : (q + 1) * QD],
                    )
                for g in range(KT // TG):
                    ps = tp_psum.tile([P, TG * P], FP, tag="tp", name="psx")
                    for j in range(TG):
                        k = g * TG + j
                        nc.tensor.transpose(
                            ps[:, j * P : (j + 1) * P],
                            xraw[:, k * P : (k + 1) * P],
                            ident[:],
                        )
                    nc.vector.tensor_copy(
                        xt_all[:, g * TG : (g + 1) * TG, tsl],
                        ps[:].rearrange("p (j q) -> p j q", j=TG),
                    )
                if tb % TBN == TBN - 1:
                    # U chunk for these token-blocks: U[r, nsl] = A @ x^T
                    n = tb // TBN
                    nsl = slice(n * NCH, (n + 1) * NCH)
                    ups = ot_psum.tile([r, NCH], FP, tag="ot", name="ups")
                    for k in range(KT):
                        nc.tensor.matmul(
                            ups[:],
                            at_sb[:, k, :],
                            xt_all[:, k, nsl],
                            start=(k == 0),
                            stop=(k == KT - 1),
                        )
                    nc.vector.tensor_copy(u_sb[:, nsl], ups[:])

        # ---- stage 3: main loop; next block's W transposes interleave with
        # the current block's matmuls so weight loads hide under them ----
        wraw_pool = ctx.enter_context(tc.tile_pool(name="wrawp", bufs=2))
        wt_pool = ctx.enter_context(tc.tile_pool(name="wtp", bufs=2))
        mm_psum = ctx.enter_context(tc.tile_pool(name="mmps", bufs=2, space="PSUM"))
        ob_pool = ctx.enter_context(tc.tile_pool(name="obp", bufs=2))
        ot_pool = ctx.enter_context(tc.tile_pool(name="otp", bufs=3))

        wstate = {}

        def w_step(m_next, k):
            """Emit DMA/transpose/evict steps for tile k of W row-block m_next."""
            if k % HKT == 0:
                h = k // HKT
                wraw = wraw_pool.tile([P, HKT * P], FP, tag="wraw", name="wraw")
                nc.sync.dma_start(
                    wraw[:],
                    w_d[m_next * P : (m_next + 1) * P, h * HKT * P : (h + 1) * HKT * P],
                )
                wstate["wraw"] = wraw
            if k % TG == 0:
                wstate["ps"] = tp_psum.tile([P, TG * P], FP, tag="tp", name="psw")
            kk = k % HKT
            nc.tensor.transpose(
                wstate["ps"][:, (k % TG) * P : (k % TG + 1) * P],
                wstate["wraw"][:, kk * P : (kk + 1) * P],
                ident[:],
            )
            if k % TG == TG - 1:
                g = k // TG
                nc.vector.tensor_copy(
                    wstate["wt"][:, g * TG : (g + 1) * TG, :],
                    wstate["ps"][:].rearrange("p (j q) -> p j q", j=TG),
                )

        # prologue: build wt for m=0
        wstate["wt"] = wt_pool.tile([P, KT, P], FPR, tag="wt", name="wt0")
        for k in range(KT):
            w_step(0, k)

        for m in range(MT):
            msl = slice(m * P, (m + 1) * P)
            wt_cur = wstate["wt"]
            if m + 1 < MT:
                wstate["wt"] = wt_pool.tile([P, KT, P], FPR, tag="wt", name="wtn")

            mps = [
                mm_psum.tile([P, NCH], FP, tag=f"mm{n}", name=f"mps{n}")
                for n in range(NT)
            ]
            for k in range(KT):
                if m + 1 < MT:
                    w_step(m + 1, k)
                for n in range(NT):
                    nc.tensor.matmul(
                        mps[n][:],
                        wt_cur[:, k, :],
                        xt_all[:, k, n * NCH : (n + 1) * NCH],
                        start=(k == 0),
                        stop=False,
                    )
            for n in range(NT):
                nsl = slice(n * NCH, (n + 1) * NCH)
                nc.tensor.matmul(
                    mps[n][:],
                    bt_sb[:r, msl],
                    u_sb[:r, nsl],
                    start=False,
                    stop=True,
                )
                ob = ob_pool.tile([P, NCH], FP, tag="ob")
                nc.scalar.activation(
                    ob[:],
                    mps[n][:],
                    mybir.ActivationFunctionType.Identity,
                    bias=bias_sb[:, m : m + 1],
                )
                otps = ot_psum.tile([P, NCH], FP, tag="ot", name="otps")
                for j in range(NCH // P):
                    nc.tensor.transpose(
                        otps[:, j * P : (j + 1) * P],
                        ob[:, j * P : (j + 1) * P],
                        ident[:],
                    )
                ot = ot_pool.tile([P, NCH], FP, tag="otsb")
                nc.vector.tensor_copy(ot[:], otps[:])
                dst = out_d[nsl, msl].rearrange("(j p) o -> p j o", p=P)
                nc.sync.dma_start(dst, ot[:].rearrange("p (j o) -> p j o", o=P))

    nc.compile()
    return nc


def make_in_maps(x, weight, bias, lora_A, lora_B):
    xf = np.ascontiguousarray(x.reshape(-1, x.shape[-1]), dtype=np.float32)
    T = xf.shape[0] // N_CORES
    MT = weight.shape[0] // P
    w = np.ascontiguousarray(weight, dtype=np.float32)
    bias_r = np.ascontiguousarray(
        bias.astype(np.float32).reshape(MT, P).T
    )
    at = np.ascontiguousarray(lora_A.astype(np.float32).T)
    bt = np.ascontiguousarray(lora_B.astype(np.float32).T)
    return [
        {
            "x": np.ascontiguousarray(xf[c * T : (c + 1) * T]),
            "w": w,
            "bias_r": bias_r,
            "at": at,
            "bt": bt,
        }
        for c in range(N_CORES)
    ]


_nc_cache = {}


def kernel(x, weight, bias, lora_A, lora_B):
    key = (x.shape, weight.shape)
    if key not in _nc_cache:
        _nc_cache[key] = build_nc()
    nc = _nc_cache[key]
    in_maps = make_in_maps(x, weight, bias, lora_A, lora_B)
    res = bass_utils.run_bass_kernel_spmd(
        nc, in_maps, core_ids=list(range(N_CORES))
    )
    out = np.concatenate([res.results[c]["out"] for c in range(N_CORES)], axis=0)
    return out.reshape(x.shape[:-1] + (weight.shape[0],))


if __name__ == "__main__":
    rng = np.random.default_rng(0)
    x = rng.standard_normal((B_FULL, S_FULL, D_IN), dtype=np.float32)
    w = (rng.standard_normal((D_OUT, D_IN), dtype=np.float32) * 0.02).astype(np.float32)
    b = (rng.standard_normal((D_OUT,), dtype=np.float32) * 0.02).astype(np.float32)
    la = (rng.standard_normal((R, D_IN), dtype=np.float32) * 0.02).astype(np.float32)
    lb = (rng.standard_normal((D_OUT, R), dtype=np.float32) * 0.02).astype(np.float32)
    out = kernel(x, w, b, la, lb)
    ref = x.reshape(-1, D_IN) @ (w + lb @ la).T + b
    err = np.abs(out.reshape(-1, D_OUT) - ref)
    denom = np.abs(ref).max()
    print("max abs err:", err.max(), "rel:", err.max() / denom)


# revision 10
# speedup vs baseline: 1.0208x; 1.0208x over previous
"""LoRA linear layer (out = x @ (W + B@A).T + bias) on 8 trn2 NeuronCores.

Strategy: data-parallel over tokens (B*S = 8192 -> 1024 tokens/core).
Each core computes its token-shard against the full weight:
  - x shard is DMA'd in, transposed on the PE (128x128 tiles) into a
    resident SBUF xT [d_in, T] laid out as [128, KT, T].
  - U = (x @ A.T).T = [r, T] computed once with A.T as stationary operand.
  - For each 128-row block m of W: DMA the rows, PE-transpose into
    WT [128, KT, 128]; accumulate psum[o=128, t=512] over the 32 k-tiles
    (fp32r matmuls, weight loaded once per (m,k) and reused for both
    t-chunks), then one rank-16 matmul adds the LoRA term from B.T and U.
  - psum is evicted through the Scalar engine with the bias added
    (bias laid out per-partition), PE-transposed back to [t, o] tiles and
    DMA'd out contiguously.
"""

import sys
import types

sys.path.insert(0, "/opt/trn_rl_repo")

import numpy as np

import concourse.bass as bass  # noqa: F401
import concourse.bacc as bacc
import concourse.tile as tile
from concourse import mybir, bass_utils
from concourse.masks import make_identity
from contextlib import ExitStack

P = 128
N_CORES = 8

# Full problem shapes (hardcoded per contract).
B_FULL, S_FULL, D_IN, D_OUT, R = 4, 2048, 4096, 4096, 16
T_CORE = (B_FULL * S_FULL) // N_CORES  # 1024 tokens per core


def build_nc(T=T_CORE, DIN=D_IN, DOUT=D_OUT, r=R, tr_fpr=False, w_bf16=False):
    """Build the per-core bass program. All cores run the same program on
    different token shards."""
    FP = mybir.dt.float32
    FPR = mybir.dt.float32r
    KT = DIN // P
    MT = DOUT // P
    NCH = min(512, T)  # moving-operand chunk (>=256 keeps fp32r at full rate)
    NT = T // NCH
    TG = 4  # transposes grouped per PSUM bank before one batched eviction
    NWCH = 4  # W row-block DMA'd in this many chunks
    HKT = KT // NWCH

    nc = bacc.Bacc("TRN2", target_bir_lowering=False, debug=False)
    x_d = nc.dram_tensor("x", [T, DIN], FP, kind="ExternalInput").ap()
    w_d = nc.dram_tensor("w", [DOUT, DIN], FP, kind="ExternalInput").ap()
    br_d = nc.dram_tensor("bias_r", [P, MT], FP, kind="ExternalInput").ap()
    at_d = nc.dram_tensor("at", [DIN, r], FP, kind="ExternalInput").ap()
    bt_d = nc.dram_tensor("bt", [r, DOUT], FP, kind="ExternalInput").ap()
    out_d = nc.dram_tensor("out", [T, DOUT], FP, kind="ExternalOutput").ap()

    with tile.TileContext(nc) as tc, ExitStack() as ctx:
        const = ctx.enter_context(tc.tile_pool(name="const", bufs=1))
        ident = const.tile([P, P], FP)
        make_identity(nc, ident)
        bias_sb = const.tile([P, MT], FP)
        nc.sync.dma_start(bias_sb[:], br_d[:])
        at_sb = const.tile([P, KT, r], FPR)
        bt_sb = const.tile([r, DOUT], FPR)
        xt_all = const.tile([P, KT, T], FPR)  # resident x^T, 16 MB
        u_sb = const.tile([r, T], FPR)

        tp_psum = ctx.enter_context(tc.tile_pool(name="tpps", bufs=3, space="PSUM"))
        ot_psum = ctx.enter_context(tc.tile_pool(name="otps", bufs=1, space="PSUM"))

        # ---- stage 1: transpose x shard into xt_all (TG tiles per bank) ----
        with tc.tile_pool(name="xrawp", bufs=2) as xraw_pool:
            at_raw = xraw_pool.tile([P, KT, r], FP, tag="xraw")
            nc.sync.dma_start(at_raw[:], at_d.rearrange("(k p) r -> p k r", p=P))
            nc.vector.tensor_copy(at_sb[:], at_raw[:])
            bt_raw = xraw_pool.tile([r, DOUT], FP, tag="xraw")
            nc.sync.dma_start(bt_raw[:], bt_d[:])
            nc.vector.tensor_copy(bt_sb[:], bt_raw[:])
            TBN = NCH // P  # token-blocks per U chunk
            for tb in range(T // P):
                tsl = slice(tb * P, (tb + 1) * P)
                xraw = xraw_pool.tile([P, DIN], FP, tag="xraw")
                QD = DIN // 4
                for q in range(4):
                    nc.sync.dma_start(
                        xraw[:, q * QD : (q + 1) * QD],
                        x_d[tsl, q * QD : (q + 1) * QD],
                    )
                for g in range(KT // TG):
                    ps = tp_psum.tile([P, TG * P], FP, tag="tp", name="psx")
                    for j in range(TG):
                        k = g * TG + j
                        nc.tensor.transpose(
                            ps[:, j * P : (j + 1) * P],
                            xraw[:, k * P : (k + 1) * P],
                            ident[:],
                        )
                    nc.vector.tensor_copy(
                        xt_all[:, g * TG : (g + 1) * TG, tsl],
                        ps[:].rearrange("p (j q) -> p j q", j=TG),
                    )
                if tb % TBN == TBN - 1:
                    # U chunk for these token-blocks: U[r, nsl] = A @ x^T
                    n = tb // TBN
                    nsl = slice(n * NCH, (n + 1) * NCH)
                    ups = ot_psum.tile([r, NCH], FP, tag="ot", name="ups")
                    for k in range(KT):
                        nc.tensor.matmul(
                            ups[:],
                            at_sb[:, k, :],
                            xt_all[:, k, nsl],
                            start=(k == 0),
                            stop=(k == KT - 1),
                        )
                    nc.vector.tensor_copy(u_sb[:, nsl], ups[:])

        # ---- stage 3: main loop; next block's W transposes interleave with
        # the current block's matmuls so weight loads hide under them ----
        wraw_pool = ctx.enter_context(tc.tile_pool(name="wrawp", bufs=2))
        wt_pool = ctx.enter_context(tc.tile_pool(name="wtp", bufs=2))
        mm_psum = ctx.enter_context(tc.tile_pool(name="mmps", bufs=2, space="PSUM"))
        ob_pool = ctx.enter_context(tc.tile_pool(name="obp", bufs=2))
        ot_pool = ctx.enter_context(tc.tile_pool(name="otp", bufs=3))

        wstate = {}

        def w_step(m_next, k):
            """Emit DMA/transpose/evict steps for tile k of W row-block m_next."""
            if k % HKT == 0:
                h = k // HKT
                wraw = wraw_pool.tile([P, HKT * P], FP, tag="wraw", name="wraw")
                nc.sync.dma_start(
                    wraw[:],
                    w_d[m_next * P : (m_next + 1) * P, h * HKT * P : (h + 1) * HKT * P],
                )
                wstate["wraw"] = wraw
            if k % TG == 0:
                wstate["ps"] = tp_psum.tile([P, TG * P], FP, tag="tp", name="psw")
            kk = k % HKT
            nc.tensor.transpose(
                wstate["ps"][:, (k % TG) * P : (k % TG + 1) * P],
                wstate["wraw"][:, kk * P : (kk + 1) * P],
                ident[:],
            )
            if k % TG == TG - 1:
                g = k // TG
                nc.vector.tensor_copy(
                    wstate["wt"][:, g * TG : (g + 1) * TG, :],
                    wstate["ps"][:].rearrange("p (j q) -> p j q", j=TG),
                )

        # prologue: build wt for m=0
        wstate["wt"] = wt_pool.tile([P, KT, P], FPR, tag="wt", name="wt0")
        for k in range(KT):
            w_step(0, k)

        for m in range(MT):
            msl = slice(m * P, (m + 1) * P)
            wt_cur = wstate["wt"]
            if m + 1 < MT:
                wstate["wt"] = wt_pool.tile([P, KT, P], FPR, tag="wt", name="wtn")

            mps = [
                mm_psum.tile([P, NCH], FP, tag=f"mm{n}", name=f"mps{n}")
                for n in range(NT)
            ]
            for k in range(KT):
                if m + 1 < MT:
                    w_step(m + 1, k)
                for n in range(NT):
                    nc.tensor.matmul(
                        mps[n][:],
                        wt_cur[:, k, :],
                        xt_all[:, k, n * NCH : (n + 1) * NCH],
                        start=(k == 0),
                        stop=False,
                    )
            for n in range(NT):
                nsl = slice(n * NCH, (n + 1) * NCH)
                nc.tensor.matmul(
                    mps[n][:],
                    bt_sb[:r, msl],
                    u_sb[:r, nsl],
                    start=False,
                    stop=True,
                )
                ob = ob_pool.tile([P, NCH], FP, tag="ob")
                nc.scalar.activation(
                    ob[:],
                    mps[n][:],
                    mybir.ActivationFunctionType.Identity,
                    bias=bias_sb[:, m : m + 1],
                )
                otps = ot_psum.tile([P, NCH], FP, tag="ot", name="otps")
                for j in range(NCH // P):
                    nc.tensor.transpose(
                        otps[:, j * P : (j + 1) * P],
                        ob[:, j * P : (j + 1) * P],
                        ident[:],
                    )
                ot = ot_pool.tile([P, NCH], FP, tag="otsb")
                nc.vector.tensor_copy(ot[:], otps[:])
                dst = out_d[nsl, msl].rearrange("(j p) o -> p j o", p=P)
                nc.sync.dma_start(dst, ot[:].rearrange("p (j o) -> p j o", o=P))

    nc.compile()
    return nc


def make_in_maps(x, weight, bias, lora_A, lora_B):
    xf = np.ascontiguousarray(x.reshape(-1, x.shape[-1]), dtype=np.float32)
    T = xf.shape[0] // N_CORES
    MT = weight.shape[0] // P
    w = np.ascontiguousarray(weight, dtype=np.float32)
    bias_r = np.ascontiguousarray(
        bias.astype(np.float32).reshape(MT, P).T
    )
    at = np.ascontiguousarray(lora_A.astype(np.float32).T)
    bt = np.ascontiguousarray(lora_B.astype(np.float32).T)
    return [
        {
            "x": np.ascontiguousarray(xf[c * T : (c + 1) * T]),
            "w": w,
            "bias_r": bias_r,
            "at": at,
            "bt": bt,
        }
        for c in range(N_CORES)
    ]


_nc_cache = {}


def kernel(x, weight, bias, lora_A, lora_B):
    key = (x.shape, weight.shape)
    if key not in _nc_cache:
        _nc_cache[key] = build_nc()
    nc = _nc_cache[key]
    in_maps = make_in_maps(x, weight, bias, lora_A, lora_B)
    res = bass_utils.run_bass_kernel_spmd(
        nc, in_maps, core_ids=list(range(N_CORES))
    )
    out = np.concatenate([res.results[c]["out"] for c in range(N_CORES)], axis=0)
    return out.reshape(x.shape[:-1] + (weight.shape[0],))


if __name__ == "__main__":
    rng = np.random.default_rng(0)
    x = rng.standard_normal((B_FULL, S_FULL, D_IN), dtype=np.float32)
    w = (rng.standard_normal((D_OUT, D_IN), dtype=np.float32) * 0.02).astype(np.float32)
    b = (rng.standard_normal((D_OUT,), dtype=np.float32) * 0.02).astype(np.float32)
    la = (rng.standard_normal((R, D_IN), dtype=np.float32) * 0.02).astype(np.float32)
    lb = (rng.standard_normal((D_OUT, R), dtype=np.float32) * 0.02).astype(np.float32)
    out = kernel(x, w, b, la, lb)
    ref = x.reshape(-1, D_IN) @ (w + lb @ la).T + b
    err = np.abs(out.reshape(-1, D_OUT) - ref)
    denom = np.abs(ref).max()
    print("max abs err:", err.max(), "rel:", err.max() / denom)


# revision 11
# speedup vs baseline: 1.0514x; 1.0299x over previous
"""LoRA linear layer (out = x @ (W + B@A).T + bias) on 8 trn2 NeuronCores.

Strategy: data-parallel over tokens (B*S = 8192 -> 1024 tokens/core).
Each core computes its token-shard against the full weight:
  - x shard is DMA'd in, transposed on the PE (128x128 tiles) into a
    resident SBUF xT [d_in, T] laid out as [128, KT, T].
  - U = (x @ A.T).T = [r, T] computed once with A.T as stationary operand.
  - For each 128-row block m of W: DMA the rows, PE-transpose into
    WT [128, KT, 128]; accumulate psum[o=128, t=512] over the 32 k-tiles
    (fp32r matmuls, weight loaded once per (m,k) and reused for both
    t-chunks), then one rank-16 matmul adds the LoRA term from B.T and U.
  - psum is evicted through the Scalar engine with the bias added
    (bias laid out per-partition), PE-transposed back to [t, o] tiles and
    DMA'd out contiguously.
"""

import sys
import types

sys.path.insert(0, "/opt/trn_rl_repo")

import numpy as np

import concourse.bass as bass  # noqa: F401
import concourse.bacc as bacc
import concourse.tile as tile
from concourse import mybir, bass_utils
from concourse.masks import make_identity
from contextlib import ExitStack

P = 128
N_CORES = 8

# Full problem shapes (hardcoded per contract).
B_FULL, S_FULL, D_IN, D_OUT, R = 4, 2048, 4096, 4096, 16
T_CORE = (B_FULL * S_FULL) // N_CORES  # 1024 tokens per core


def build_nc(T=T_CORE, DIN=D_IN, DOUT=D_OUT, r=R, tr_fpr=False, w_bf16=False):
    """Build the per-core bass program. All cores run the same program on
    different token shards."""
    FP = mybir.dt.float32
    FPR = mybir.dt.float32r
    KT = DIN // P
    MT = DOUT // P
    NCH = min(512, T)  # moving-operand chunk (>=256 keeps fp32r at full rate)
    NT = T // NCH
    TG = 4  # transposes grouped per PSUM bank before one batched eviction
    NWCH = 4  # W row-block DMA'd in this many chunks
    HKT = KT // NWCH

    nc = bacc.Bacc("TRN2", target_bir_lowering=False, debug=False)
    x_d = nc.dram_tensor("x", [T, DIN], FP, kind="ExternalInput").ap()
    w_d = nc.dram_tensor("w", [DOUT, DIN], FP, kind="ExternalInput").ap()
    br_d = nc.dram_tensor("bias_r", [P, MT], FP, kind="ExternalInput").ap()
    at_d = nc.dram_tensor("at", [DIN, r], FP, kind="ExternalInput").ap()
    bt_d = nc.dram_tensor("bt", [r, DOUT], FP, kind="ExternalInput").ap()
    out_d = nc.dram_tensor("out", [T, DOUT], FP, kind="ExternalOutput").ap()

    with tile.TileContext(nc) as tc, ExitStack() as ctx:
        const = ctx.enter_context(tc.tile_pool(name="const", bufs=1))
        ident = const.tile([P, P], FP)
        make_identity(nc, ident)
        bias_sb = const.tile([P, MT], FP)
        nc.sync.dma_start(bias_sb[:], br_d[:])
        at_sb = const.tile([P, KT, r], FPR)
        bt_sb = const.tile([r, DOUT], FPR)
        xt_all = const.tile([P, KT, T], FPR)  # resident x^T, 16 MB
        u_sb = const.tile([r, T], FPR)

        tp_psum = ctx.enter_context(tc.tile_pool(name="tpps", bufs=3, space="PSUM"))
        ot_psum = ctx.enter_context(tc.tile_pool(name="otps", bufs=1, space="PSUM"))

        # ---- stage 1: transpose x shard into xt_all (TG tiles per bank) ----
        with tc.tile_pool(name="xrawp", bufs=3) as xraw_pool:
            at_raw = xraw_pool.tile([P, KT, r], FP, tag="xraw")
            nc.sync.dma_start(at_raw[:], at_d.rearrange("(k p) r -> p k r", p=P))
            nc.vector.tensor_copy(at_sb[:], at_raw[:])
            bt_raw = xraw_pool.tile([r, DOUT], FP, tag="xraw")
            nc.sync.dma_start(bt_raw[:], bt_d[:])
            nc.vector.tensor_copy(bt_sb[:], bt_raw[:])
            TBN = NCH // P  # token-blocks per U chunk
            for tb in range(T // P):
                tsl = slice(tb * P, (tb + 1) * P)
                xraw = xraw_pool.tile([P, DIN], FP, tag="xraw")
                QD = DIN // 4
                for q in range(4):
                    nc.sync.dma_start(
                        xraw[:, q * QD : (q + 1) * QD],
                        x_d[tsl, q * QD : (q + 1) * QD],
                    )
                for g in range(KT // TG):
                    ps = tp_psum.tile([P, TG * P], FP, tag="tp", name="psx")
                    for j in range(TG):
                        k = g * TG + j
                        nc.tensor.transpose(
                            ps[:, j * P : (j + 1) * P],
                            xraw[:, k * P : (k + 1) * P],
                            ident[:],
                        )
                    nc.vector.tensor_copy(
                        xt_all[:, g * TG : (g + 1) * TG, tsl],
                        ps[:].rearrange("p (j q) -> p j q", j=TG),
                    )
                if tb % TBN == TBN - 1:
                    # U chunk for these token-blocks: U[r, nsl] = A @ x^T
                    n = tb // TBN
                    nsl = slice(n * NCH, (n + 1) * NCH)
                    ups = ot_psum.tile([r, NCH], FP, tag="ot", name="ups")
                    for k in range(KT):
                        nc.tensor.matmul(
                            ups[:],
                            at_sb[:, k, :],
                            xt_all[:, k, nsl],
                            start=(k == 0),
                            stop=(k == KT - 1),
                        )
                    nc.vector.tensor_copy(u_sb[:, nsl], ups[:])

        # ---- stage 3: main loop; next block's W transposes interleave with
        # the current block's matmuls so weight loads hide under them ----
        wraw_pool = ctx.enter_context(tc.tile_pool(name="wrawp", bufs=3))
        wt_pool = ctx.enter_context(tc.tile_pool(name="wtp", bufs=2))
        mm_psum = ctx.enter_context(tc.tile_pool(name="mmps", bufs=2, space="PSUM"))
        ob_pool = ctx.enter_context(tc.tile_pool(name="obp", bufs=3))
        ot_pool = ctx.enter_context(tc.tile_pool(name="otp", bufs=3))

        wstate = {}

        def w_step(m_next, k):
            """Emit DMA/transpose/evict steps for tile k of W row-block m_next."""
            if k % HKT == 0:
                h = k // HKT
                wraw = wraw_pool.tile([P, HKT * P], FP, tag="wraw", name="wraw")
                nc.sync.dma_start(
                    wraw[:],
                    w_d[m_next * P : (m_next + 1) * P, h * HKT * P : (h + 1) * HKT * P],
                )
                wstate["wraw"] = wraw
            if k % TG == 0:
                wstate["ps"] = tp_psum.tile([P, TG * P], FP, tag="tp", name="psw")
            kk = k % HKT
            nc.tensor.transpose(
                wstate["ps"][:, (k % TG) * P : (k % TG + 1) * P],
                wstate["wraw"][:, kk * P : (kk + 1) * P],
                ident[:],
            )
            if k % TG == TG - 1:
                g = k // TG
                nc.vector.tensor_copy(
                    wstate["wt"][:, g * TG : (g + 1) * TG, :],
                    wstate["ps"][:].rearrange("p (j q) -> p j q", j=TG),
                )

        # prologue: build wt for m=0
        wstate["wt"] = wt_pool.tile([P, KT, P], FPR, tag="wt", name="wt0")
        for k in range(KT):
            w_step(0, k)

        for m in range(MT):
            msl = slice(m * P, (m + 1) * P)
            wt_cur = wstate["wt"]
            if m + 1 < MT:
                wstate["wt"] = wt_pool.tile([P, KT, P], FPR, tag="wt", name="wtn")

            mps = [
                mm_psum.tile([P, NCH], FP, tag=f"mm{n}", name=f"mps{n}")
                for n in range(NT)
            ]
            for k in range(KT):
                if m + 1 < MT:
                    w_step(m + 1, k)
                for n in range(NT):
                    nc.tensor.matmul(
                        mps[n][:],
                        wt_cur[:, k, :],
                        xt_all[:, k, n * NCH : (n + 1) * NCH],
                        start=(k == 0),
                        stop=False,
                    )
            for n in range(NT):
                nsl = slice(n * NCH, (n + 1) * NCH)
                nc.tensor.matmul(
                    mps[n][:],
                    bt_sb[:r, msl],
                    u_sb[:r, nsl],
                    start=False,
                    stop=True,
                )
                ob = ob_pool.tile([P, NCH], FP, tag="ob")
                nc.scalar.activation(
                    ob[:],
                    mps[n][:],
                    mybir.ActivationFunctionType.Identity,
                    bias=bias_sb[:, m : m + 1],
                )
                otps = ot_psum.tile([P, NCH], FP, tag="ot", name="otps")
                for j in range(NCH // P):
                    nc.tensor.transpose(
                        otps[:, j * P : (j + 1) * P],
                        ob[:, j * P : (j + 1) * P],
                        ident[:],
                    )
                ot = ot_pool.tile([P, NCH], FP, tag="otsb")
                nc.vector.tensor_copy(ot[:], otps[:])
                dst = out_d[nsl, msl].rearrange("(j p) o -> p j o", p=P)
                nc.sync.dma_start(dst, ot[:].rearrange("p (j o) -> p j o", o=P))

    nc.compile()
    return nc


def make_in_maps(x, weight, bias, lora_A, lora_B):
    xf = np.ascontiguousarray(x.reshape(-1, x.shape[-1]), dtype=np.float32)
    T = xf.shape[0] // N_CORES
    MT = weight.shape[0] // P
    w = np.ascontiguousarray(weight, dtype=np.float32)
    bias_r = np.ascontiguousarray(
        bias.astype(np.float32).reshape(MT, P).T
    )
    at = np.ascontiguousarray(lora_A.astype(np.float32).T)
    bt = np.ascontiguousarray(lora_B.astype(np.float32).T)
    return [
        {
            "x": np.ascontiguousarray(xf[c * T : (c + 1) * T]),
            "w": w,
            "bias_r": bias_r,
            "at": at,
            "bt": bt,
        }
        for c in range(N_CORES)
    ]


_nc_cache = {}


def kernel(x, weight, bias, lora_A, lora_B):
    key = (x.shape, weight.shape)
    if key not in _nc_cache:
        _nc_cache[key] = build_nc()
    nc = _nc_cache[key]
    in_maps = make_in_maps(x, weight, bias, lora_A, lora_B)
    res = bass_utils.run_bass_kernel_spmd(
        nc, in_maps, core_ids=list(range(N_CORES))
    )
    out = np.concatenate([res.results[c]["out"] for c in range(N_CORES)], axis=0)
    return out.reshape(x.shape[:-1] + (weight.shape[0],))


if __name__ == "__main__":
    rng = np.random.default_rng(0)
    x = rng.standard_normal((B_FULL, S_FULL, D_IN), dtype=np.float32)
    w = (rng.standard_normal((D_OUT, D_IN), dtype=np.float32) * 0.02).astype(np.float32)
    b = (rng.standard_normal((D_OUT,), dtype=np.float32) * 0.02).astype(np.float32)
    la = (rng.standard_normal((R, D_IN), dtype=np.float32) * 0.02).astype(np.float32)
    lb = (rng.standard_normal((D_OUT, R), dtype=np.float32) * 0.02).astype(np.float32)
    out = kernel(x, w, b, la, lb)
    ref = x.reshape(-1, D_IN) @ (w + lb @ la).T + b
    err = np.abs(out.reshape(-1, D_OUT) - ref)
    denom = np.abs(ref).max()
    print("max abs err:", err.max(), "rel:", err.max() / denom)
